# revision 19
# baseline (speedup 1.0000x reference)
"""Trainium2 Bass kernel for nn_MultiHeadSelfAttention_55654186222044.

Reference math (per batch b, per "slice" h of the reshaped activations):
    xs  = x[b,:,h*64:(h+1)*64]                  (T=1024, D=64)
    q_i = xs @ Wq[i].T + bq[i]   (per param set i=0..15), same k_i, v_i
    scores_i = q_i.T @ k_i / 8   (64x64, contraction over T!)
    w_i = softmax(scores_i, axis=-1)
    o_i = v_i @ w_i.T ;  cat = concat_i o_i     (T, 1024)
    out[b,h] = cat @ Wf.T + bf                  (T, 1024)

Because attention is over the feature dim, everything collapses through a
65x65 Gram matrix G = xa.T @ xa (xa = [xs, 1]):
    P      = G @ W~k_all                         (65, 1024)
    scT_c  = P_c.T @ W~q_c   (128-col chunks)    diag 64x64 blocks = scores_i^T
             (softmax axis lands on the psum partition dim)
    exp    -> expC, stored into a PRE-ZEROED (128,8,128) tile so that each
             expC[:,c,:] is the block-diagonal [exp_2c, exp_2c+1]
    M~_c   = expC[:,c,:].T @ wva2[:,c,:]  (one matmul per chunk; col 65 of
             wva2 is ones so col 65 of M~ is the softmax denominator)
    M      = M~ * (1/denom) per row; N = M.T @ Wf.T + u x bf   (65, 1024)
    out[b,h] = xa @ N
This cuts FLOPs ~10x vs the naive dataflow. |scores| < ~50 so exp needs no
max-subtraction (f32 psum, bf16 expC storage - e^50 overflows fp16).

Schedule: the PE duty cycle is governed by a HW activity monitor - it
starts at 4/8 (half rate), is promoted to 8/8 after a ~3.4us window of
dense activity, and demoted again by idle gaps. So the kernel runs in two
phases. Phase A: heads 0+1 interleaved (their cross-engine cascades
overlap) with dummy matmul bridges between items keeping the PE activity
window dense, all under the input-DMA shadow; gpsimd (which cannot touch
PSUM and is otherwise idle) does the warmup/expC memsets so vector/scalar
start clean. Phase B: a gapless weave of N-halves and out-halves (each
out-half interleaves 1:1 with the next N-half so vector/scalar psum
evictions never locally saturate), with heads 2+3 pulled in as paced
filler. Inputs stream on the sync queue in deadline order (xh01, wqk,
ubf, wft-left, wva2, xh23, xt, wft-right); the tail half-order
(0,0),(1,0),(0,1),(1,1),... never waits on a late input byte. Softmax
normalization is all-vector so the in-order scalar queue never
head-of-line blocks on a cross-engine chain.

Sharding: 32 independent (b, h) slices; 8 cores x 4 slices. Core c takes
b = c//4 and heads 4*(c%4)..4*(c%4)+3 so its x columns are contiguous.
Weights replicated, no collectives. Output is stored fp16, partition-major
[j, p, c, :]; the host transposes back and upcasts.
"""

import numpy as np
import ml_dtypes

B, T, E, H = 2, 1024, 1024, 16
D = E // H
SCALE = float(np.sqrt(D))
NCORES = 8

_CACHE = {}


def _build_nc():
    from contextlib import ExitStack

    import concourse.mybir as mybir
    import concourse.tile as tile
    from concourse import bacc

    dt = mybir.dt
    AF = mybir.ActivationFunctionType

    nc = bacc.Bacc(None)
    xh_d = nc.declare_dram_parameter("xh", [128, 4, 8, 65], dt.float16, False)
    xt_d = nc.declare_dram_parameter("xt", [65, 4, 1024], dt.float16, False)
    wqk_d = nc.declare_dram_parameter("wqk", [65, 2048], dt.float16, False)
    ubf_d = nc.declare_dram_parameter("ubf", [1, 1089], dt.float16, False)
    wva2_d = nc.declare_dram_parameter("wva2", [128, 8, 66], dt.bfloat16, False)
    wft_d = nc.declare_dram_parameter("wft", [128, 8, 1024], dt.float16, False)
    # out stored partition-major: out_d[j, p, c, :] = out row c*128+p of slice
    # j. 1KB-contiguous per-partition DMA descriptor runs; host transposes.
    out_d = nc.declare_dram_parameter("out", [4, 128, 8, 1024], dt.float16, True)

    with ExitStack() as ctx:
        tc = ctx.enter_context(tile.TileContext(nc))
        consts = ctx.enter_context(tc.tile_pool(name="consts", bufs=1))
        sbp = ctx.enter_context(tc.tile_pool(name="sbp", bufs=4))
        outp = ctx.enter_context(tc.tile_pool(name="outp", bufs=8))
        # PSUM (8 banks): ph 3 (head ring: gps/pps/scp/mall), pn 2 (N halves),
        # po 3 (out-stage ring + warmup dummies).
        ph = ctx.enter_context(tc.tile_pool(name="ph", bufs=3, space="PSUM"))
        pn = ctx.enter_context(tc.tile_pool(name="pn", bufs=2, space="PSUM"))
        po = ctx.enter_context(tc.tile_pool(name="po", bufs=3, space="PSUM"))

        # ---- input DMAs, single sync queue, deadline order: head-critical
        # bytes first, then wft-left (phase B gate), then the rest ----
        xh = consts.tile([128, 4, 8, 65], dt.float16, name="xh")
        nc.sync.dma_start(out=xh[:, 0:2], in_=xh_d[:, 0:2])
        wqk = consts.tile([65, 2048], dt.float16, name="wqk")
        nc.sync.dma_start(out=wqk[:], in_=wqk_d[:, :])
        ubf = consts.tile([1, 1089], dt.float16, name="ubf")
        nc.sync.dma_start(out=ubf[:], in_=ubf_d[:, :])
        wft = consts.tile([128, 8, 1024], dt.float16, name="wft")
        nc.sync.dma_start(out=wft[:, :, 0:512], in_=wft_d[:, :, 0:512])
        wva2 = consts.tile([128, 8, 66], dt.bfloat16, name="wva2")
        nc.sync.dma_start(out=wva2[:], in_=wva2_d[:, :, :])
        nc.sync.dma_start(out=xh[:, 2:4], in_=xh_d[:, 2:4])
        xt = consts.tile([65, 4, 1024], dt.float16, name="xt")
        nc.sync.dma_start(out=xt[:], in_=xt_d[:, :, :])
        nc.sync.dma_start(out=wft[:, :, 512:1024], in_=wft_d[:, :, 512:1024])

        # ---- PE warmup: ramp the clock while xh[0]/wqk land ----
        warm = consts.tile([128, 512], dt.float16, name="warm")
        nc.gpsimd.memset(warm[:], 0.0)
        wps = po.tile([128, 512], dt.float32, name="warm_ps", tag="po")
        for _ in range(12):
            nc.tensor.matmul(wps[:, 0:128], warm[:, 0:128], warm[:, 0:128],
                             start=True, stop=True)

        def bridge(n, nm):
            bps = po.tile([128, 512], dt.float32, name=f"br_{nm}", tag="po")
            for k in range(n):
                nc.tensor.matmul(bps[:, 0:128], warm[:, 0:128], warm[:, 0:128],
                                 start=True, stop=True)

        # expC buffers: pre-zero all four (gpsimd, idle at start); only the
        # diagonal 64-blocks are ever rewritten, so off-diag zeros persist.
        expC = {}
        for j in range(4):
            expC[j] = sbp.tile([128, 8, 128], dt.bfloat16, name=f"expC_{j}",
                               tag="expC")
            nc.gpsimd.memset(expC[j][:], 0.0)

        # nbf: rows 0:63 zero, row 64 = bf. Folded into the nsb eviction as a
        # tensor_tensor add, replacing the per-half (1,65)x(1,512) ub matmuls.
        nbf = consts.tile([65, 1024], dt.float16, name="nbf")
        nc.gpsimd.memset(nbf[:], 0.0)
        nc.gpsimd.tensor_copy(out=nbf[64:65, :], in_=ubf[0:1, 0:1024])

        wqt = wqk[0:65, 0:1024]
        wkt = wqk[0:65, 1024:2048]
        bfh = ubf[0:1, 0:1024]
        ub = ubf[0:1, 1024:1089]

        msb = {}
        nsb = {}

        _SENT = object()

        def head_items(j):
            # 6 PE work items; evictions are emitted with their producer so
            # engine queues stay in dependency order.
            gps = ph.tile([65, 65], dt.float32, name=f"gps_{j}", tag="ph")
            for c in range(8):
                nc.tensor.matmul(gps[:], xh[:, j, c, :], xh[:, j, c, :],
                                 start=(c == 0), stop=(c == 7))
            if j == 0:
                bridge(3, "p0")
            gsb = sbp.tile([65, 65], dt.float16, name=f"gsb_{j}", tag="gsb")
            nc.vector.tensor_copy(out=gsb[:], in_=gps[:])
            yield
            psb = sbp.tile([65, 1024], dt.float16, name=f"psb_{j}", tag="psb")
            pps0 = ph.tile([65, 512], dt.float32, name=f"pps_{j}_0", tag="ph")
            pps1 = ph.tile([65, 512], dt.float32, name=f"pps_{j}_1", tag="ph")
            nc.tensor.matmul(pps0[:], gsb[:], wkt[:, 0:512], start=True, stop=True)
            nc.tensor.matmul(pps1[:], gsb[:], wkt[:, 512:1024], start=True, stop=True)
            nc.scalar.copy(out=psb[:, 0:512], in_=pps0[:])
            nc.vector.tensor_copy(out=psb[:, 512:1024], in_=pps1[:])
            yield
            for t in range(2):
                scp = ph.tile([128, 4, 128], dt.float32, name=f"scp_{j}_{t}",
                              tag="ph")
                for u in range(4):
                    c = 4 * t + u
                    nc.tensor.matmul(
                        scp[:, u, :],
                        psb[:, c * 128:(c + 1) * 128],
                        wqt[:, c * 128:(c + 1) * 128],
                        start=True, stop=True,
                    )
                nc.scalar.activation(
                    out=expC[j][0:64, 4 * t:4 * t + 4, 0:64],
                    in_=scp[0:64, :, 0:64], func=AF.Exp)
                nc.scalar.activation(
                    out=expC[j][64:128, 4 * t:4 * t + 4, 64:128],
                    in_=scp[64:128, :, 64:128], func=AF.Exp)
                yield
            rec = sbp.tile([128, 8], dt.float32, name=f"rec_{j}", tag="rec")
            msb[j] = sbp.tile([128, 8, 65], dt.float16, name=f"msb_{j}", tag="msb")
            mall = []
            for half in range(2):
                mps = ph.tile([128, 4, 66], dt.float32, name=f"mps_{j}_{half}",
                              tag="ph")
                mall.append(mps)
                for u in range(4):
                    c = 4 * half + u
                    nc.tensor.matmul(mps[:, u, :], expC[j][:, c, :],
                                     wva2[:, c, :], start=True, stop=True)
                nc.vector.reciprocal(out=rec[:, 4 * half:4 * half + 4],
                                     in_=mps[:, :, 65])
                if half == 0:
                    yield
            for c in range(8):
                nc.vector.tensor_scalar_mul(
                    out=msb[j][:, c, :], in0=mall[c // 4][:, c % 4, 0:65],
                    scalar1=rec[:, c:c + 1])
            yield

        def N_items(j, nh):
            nsp = pn.tile([65, 512], dt.float32, name=f"nsp_{j}_{nh}", tag="pn")
            lo = nh * 512
            for c in range(8):
                nc.tensor.matmul(nsp[:], msb[j][:, c, :], wft[:, c, lo:lo + 512],
                                 start=(c == 0), stop=(c == 7))
                yield
            if nh == 0:
                nsb[j] = sbp.tile([65, 1024], dt.float16, name=f"nsb_{j}",
                                  tag="nsb")
            nc.vector.tensor_add(out=nsb[j][:, lo:lo + 512], in0=nsp[:],
                                 in1=nbf[:, lo:lo + 512])
            yield

        def out_items(j, nh):
            lo = nh * 512
            grp = {(3, 0): 2, (3, 1): 1}.get((j, nh), 4)
            osb = None
            for c in range(8):
                if c % grp == 0:
                    osb = outp.tile([128, grp, 512], dt.float16,
                                    name=f"osb_{j}_{nh}_{c // grp}", tag="osb")
                ops = po.tile([128, 512], dt.float32, name=f"ops_{j}_{nh}_{c}",
                              tag="po")
                nc.tensor.matmul(ops[:], xt[:, j, c * 128:(c + 1) * 128],
                                 nsb[j][:, lo:lo + 512], start=True, stop=True)
                if c % 3 == 1:
                    nc.vector.tensor_copy(out=osb[:, c % grp, :], in_=ops[:])
                else:
                    nc.scalar.copy(out=osb[:, c % grp, :], in_=ops[:])
                if c % grp == grp - 1:
                    nc.sync.dma_start(
                        out=out_d[j, :, c - grp + 1:c + 1, lo:lo + 512],
                        in_=osb[:])
                yield

        # h0 runs alone (gated on input DMA anyway); h1-h3 are woven into
        # the tail stream as filler so their cross-engine latency hides under
        # tail matmuls. Each out-half interleaves 1:1 with the next N-half so
        # psum evictions on vector/scalar never locally saturate.
        import itertools

        # Phase A: h0 + h1 interleaved, dummy bridges between every item so
        # the PE activity window stays dense and earns the 8/8 duty cycle
        # before the throughput phase starts (HAM governor halves the PE duty
        # after idle gaps; re-earning takes a ~3.4us probation window).
        ga, gb = head_items(0), head_items(1)
        nbr = 0
        a_live = b_live = True
        while a_live or b_live:
            if a_live:
                a_live = next(ga, _SENT) is not _SENT
                bridge(4, f"a{nbr}"); nbr += 1
            if b_live:
                b_live = next(gb, _SENT) is not _SENT
                bridge(4, f"a{nbr}"); nbr += 1

        F = itertools.chain(head_items(2), head_items(3))

        def pull(n):
            for _ in range(n):
                if next(F, _SENT) is _SENT:
                    return

        # Phase B: dense tail stream at full duty. Halves ordered so no
        # matmul waits on a late input byte (wft right half arrives ~13us).
        halves = [(0, 0), (1, 0), (0, 1), (1, 1), (2, 0), (2, 1), (3, 0), (3, 1)]
        bridge(18, "b0")  # cover the wft-left arrival if it lags h0/h1
        prev_out = None
        slot = 0
        for k, (j, nh) in enumerate(halves):
            n_gen = N_items(j, nh)
            n_live, o_live = True, prev_out is not None
            while n_live or o_live:
                if n_live:
                    n_live = next(n_gen, _SENT) is not _SENT
                    slot += 1
                    if slot % 4 == 0 and slot > 8:
                        pull(1)
                if o_live:
                    o_live = next(prev_out, _SENT) is not _SENT
                    slot += 1
                    if slot % 4 == 0 and slot > 8:
                        pull(1)
            prev_out = out_items(j, nh)
        for _ in prev_out:
            pass
        pull(99)

    nc.finalize()
    return nc


def _prep_weights(Wq, bq, Wk, bk, Wv, bv, Wf, bf):
    wqk = np.zeros((65, 2048), np.float16)
    wqk[:64, 0:1024] = (np.transpose(Wq, (2, 0, 1)).reshape(64, H * D) / SCALE
                        ).astype(np.float16)
    wqk[64, 0:1024] = (bq.reshape(H * D) / SCALE).astype(np.float16)
    wqk[:64, 1024:2048] = np.transpose(Wk, (2, 0, 1)).reshape(64, H * D
                                                              ).astype(np.float16)
    wqk[64, 1024:2048] = bk.reshape(H * D).astype(np.float16)
    ubf = np.zeros((1, 1089), np.float16)
    ubf[0, 0:1024] = bf.astype(np.float16)
    ubf[0, 1024 + 64] = 1.0
    wva_h = np.zeros((64, 16, 66), np.float32)
    wva_h[:, :, :64] = np.transpose(Wv, (1, 0, 2))
    wva_h[:, :, 64] = bv.T
    wva_h[:, :, 65] = 1.0
    wva2 = np.zeros((128, 8, 66), np.float32)
    wva2[0:64] = wva_h[:, 0::2, :]
    wva2[64:128] = wva_h[:, 1::2, :]
    wva2 = wva2.astype(ml_dtypes.bfloat16)
    wft = np.ascontiguousarray(
        Wf.T.reshape(8, 128, 1024).transpose(1, 0, 2)
    ).astype(np.float16)
    return wqk, ubf, wva2, wft


def _prep_x(xs):
    """xs (1024, 256) f32 -> xh (128, 4, 8, 65) fp16 with ones col,
    xt (65, 4, 1024) fp16 with ones row."""
    x16 = xs.astype(np.float16)
    xh = np.ones((128, 4, 8, 65), np.float16)
    xh[:, :, :, :64] = x16.reshape(8, 128, 4, 64).transpose(1, 2, 0, 3)
    xt = np.ones((65, 4, 1024), np.float16)
    xt[:64] = x16.reshape(1024, 4, 64).transpose(2, 1, 0)
    return xh, xt


def _run(inputs, trace=False, tmpdir=None):
    from concourse.bass_utils import run_bass_kernel_spmd

    if "nc" not in _CACHE:
        _CACHE["nc"] = _build_nc()
    nc = _CACHE["nc"]

    x = np.ascontiguousarray(np.asarray(inputs["x"]), dtype=np.float32)
    wqk, ubf, wva2, wft = _prep_weights(
        *(np.asarray(inputs[k], dtype=np.float32) for k in
          ("Wq", "bq", "Wk", "bk", "Wv", "bv", "Wf", "bf"))
    )
    common = dict(wqk=wqk, ubf=ubf, wva2=wva2, wft=wft)
    in_maps = []
    for c in range(NCORES):
        xs = np.ascontiguousarray(x[c // 4][:, (c % 4) * 256: (c % 4 + 1) * 256])
        xhc, xtc = _prep_x(xs)
        in_maps.append(dict(xh=xhc, xt=xtc, **common))

    res = run_bass_kernel_spmd(
        nc, in_maps, list(range(NCORES)), trace=trace, tmpdir=tmpdir
    )
    out = np.empty((B, H, T, E), np.float32)
    for c in range(NCORES):
        oc = res.results[c]["out"]  # (4, 128, 8, 1024): [j, p, cblk, :]
        oc = np.transpose(oc, (0, 2, 1, 3)).reshape(4, T, E)
        out[c // 4, 4 * (c % 4): 4 * (c % 4) + 4] = oc.astype(np.float32)
    return out, res.exec_time_ns


def kernel(**inputs) -> np.ndarray:
    out, _ = _run(inputs, trace=False)
    return out
